# revision 24
# baseline (speedup 1.0000x reference)
"""Trilinear 2x upsampling (TF v1 asymmetric coords) on 8 Trainium2 cores.

Math: for each resize axis, out[2i] = in[i] and out[2i+1] = 0.5*(in[i] +
in[i+1]) (edge-clamped).  The 3D op separates into 8 (H,W,D)-parity classes.

This kernel is HBM-bandwidth bound, so everything on-device runs in fp16
(rel err ~4e-4, far under the 2e-2 gate) and the (even,even,even) class --
which is bit-identical to the input -- never round-trips through the device:
the host writes it into the output directly from the original f32 input.
The remaining 7 classes are stored as packed quarter-resolution planes and
interleaved into the final channels-last layout on the host.

Scaling trick: with q2 = 0.25*x (exact in fp16), every class is a chain of
plain adds of q2 -- no further halving muls are needed if classes are stored
at mixed scales and the host rescales by a power of two during the f32 cast:
  sB   = q2 + q2(d+1)          = 0.5*B     (host x2)
  sCe  = q2 + q2(w+1)          = 0.5*Ce    (host x2)
  Cd   = sB + sB(w+1)          = Cd        (host x1)
  soee = q2_r + q2_{r+1}       = 0.5*oee   (host x2)
  oeo  = sB_r + sB_{r+1}       = oeo       (host x1)
  ooe  = sCe_r + sCe_{r+1}     = ooe       (host x1)
  sooo = Cd_r + Cd_{r+1}       = 2*ooo     (host x0.5)
Engine split: the six adds/row that feed downstream consumers run on DVE
in its packed-2-byte 2x mode (~2.4us per [128,96,48] add); the q2 scale
rides the otherwise-idle Act engine and the end-of-chain ooo add rides
the otherwise-idle Pool engine.  That keeps DVE's issue stream (~102us)
comfortably under the ~131us store stream (49.8 MB/core at ~400 GB/s),
so the DMA engines never wait on issue and the drain tail fully
overlaps.  Odd-class planes use four independent tiles + stores so a
slow store can't back-pressure the other three buffers -- on
bandwidth-starved cores the quartet-tile version stalled DVE mid-run
and amplified the slowdown.  Memory-bound at the fp16 roofline:
~163us/core in quiet windows, ~191us when the shared chip's HBM is
busy.  ~2.1x over the f32 interleaved baseline.

Hard-won constraints (verified on HW, do not regress):
  - DMA slices may crop leading free dims (the AP optimizer merges them
    into one contiguous run) but must keep the LAST dim whole: a
    last-dim crop lowers to per-row sub-512B descriptors at half-rate.
  - Pool tensor_add is ~5x slower than DVE (12.7us vs 2.4us per
    [128,96,48] add) and Act's mul ~3x slower than DVE's 4x-mode
    tensor_scalar: only ever give them work that nothing else consumes
    (q2 feeds DVE one hop later -- that pipeline covers Act's latency).
  - Splitting stores across two HWDGE rings does not increase HBM
    throughput; one SP store ring + Act load ring is optimal.

Sharding: input [2,96,96,48,32] -> [64 BC, 96 H, 96 W, 48 D].  SBUF
partition p = half*64 + bc (H split in two 48-row blocks): 128 partitions.
Each core owns 6 input H-rows per partition (+1 halo row).  W and D are
padded by one edge-replicated column on the host so edge clamping is free.
"""

import sys
import numpy as np

for _p in ("/opt/trn_rl_repo",):
    if _p not in sys.path:
        sys.path.insert(0, _p)

import concourse.mybir as mybir  # noqa: E402
from concourse import bass, tile  # noqa: E402
from concourse import bass_utils  # noqa: E402

F16 = mybir.dt.float16

B, C, H, W, D = 2, 32, 96, 96, 48
TH, TW, TD = 192, 192, 96
NCORES = 8
ROWS = 6            # owned input H rows per (core, half)
HP, WP, DP = ROWS + 1, W + 1, D + 1   # +halo: 7, 97, 49

_ws_ctr = [0]


def _split_multi_waits(nc):
    """The walrus in this environment accepts at most one semaphore wait per
    instruction (two on EventSemaphore).  Tile's wait assigner can attach
    more; move the extras onto EventSemaphore instructions inserted just
    before, on the same engine, preserving program order."""
    n_split = 0
    for f in nc.m.functions:
        for blk in f.blocks:
            out = []
            changed = False
            for inst in blk.instructions:
                si = inst.sync_info
                waits = list(si.on_wait) if si and si.on_wait else []
                cap = 2 if isinstance(inst, mybir.InstEventSemaphore) else 1
                if len(waits) > cap:
                    changed = True
                    n_split += 1
                    extra = waits[:-1]
                    for i in range(0, len(extra), 2):
                        _ws_ctr[0] += 1
                        ev = mybir.InstEventSemaphore(
                            name=f"ws_ev_{_ws_ctr[0]}", ins=[], outs=[])
                        ev.engine = inst.engine
                        ev.sync_info = mybir.SyncInfo(
                            on_wait=list(extra[i:i + 2]), on_update=[])
                        out.append(ev)
                    si.on_wait = [waits[-1]]
                    inst.sync_info = si
                out.append(inst)
            if changed:
                blk.instructions = out
    return n_split


def build_program():
    nc = bass.Bass()
    x = nc.dram_tensor("x", [128, HP, WP, DP], F16, kind="ExternalInput")
    # Per-class packed outputs (see module docstring for scales).
    yb = nc.dram_tensor("yb", [128, ROWS, WP, D], F16, kind="ExternalOutput")
    yc = nc.dram_tensor("yc", [128, ROWS, W, D], F16, kind="ExternalOutput")
    yd = nc.dram_tensor("yd", [128, ROWS, W, D], F16, kind="ExternalOutput")
    yo = nc.dram_tensor("yo", [128, ROWS, 4, W, D], F16, kind="ExternalOutput")

    with tile.TileContext(nc) as tc:
        with tc.tile_pool(name="pool", bufs=2) as pool:
            prev = None
            for r in range(HP):
                # q2 = 0.25*row: exact in fp16; on the otherwise-idle Act
                # engine so the DVE issue stream finishes sooner than the
                # store stream drains (DMA never waits on issue)
                q2 = pool.tile([128, WP, DP], F16, tag="q2", bufs=3,
                               name=f"q2_{r}")
                sB = pool.tile([128, WP, D], F16, tag="sB", bufs=3,
                               name=f"sB_{r}")
                if r == 0:
                    # row 0 runs in four W-chunks so the first store issues
                    # ~4us earlier -- with the DMA saturated end-to-end,
                    # every us earlier at the head is a us off the tail.
                    # Chunk 0's load rides SP's ring (its preamble finishes
                    # first); first-dim tile crops lower to one contiguous
                    # run, so chunk stores keep full descriptor efficiency.
                    bounds = (0, 25, 49, 73, WP)
                    for ci, (w0, w1) in enumerate(zip(bounds, bounds[1:])):
                        ph = pool.tile([128, w1 - w0, DP], F16,
                                       tag=f"p0{ci}", bufs=1,
                                       name=f"p0{ci}_0")
                        ld = nc.sync if ci == 0 else nc.scalar
                        ld.dma_start(out=ph, in_=x[:, 0, w0:w1, :])
                        nc.scalar.mul(q2[:, w0:w1, :], ph, 0.25)
                        nc.vector.tensor_add(sB[:, w0:w1, :],
                                             q2[:, w0:w1, 0:D],
                                             q2[:, w0:w1, 1:DP])
                        nc.sync.dma_start(out=yb[:, 0, w0:w1, :],
                                          in_=sB[:, w0:w1, :])
                else:
                    # input row (Act's HWDGE ring, so loads don't queue
                    # behind the output stores on SP's ring)
                    p = pool.tile([128, WP, DP], F16, tag="p", bufs=2,
                                  name=f"p_{r}")
                    nc.scalar.dma_start(out=p, in_=x[:, r])
                    nc.scalar.mul(q2, p, 0.25)
                    # packed adds run in DVE 2x mode (2-byte, unit stride);
                    # each even-class store issues right after its producer
                    nc.vector.tensor_add(sB, q2[:, :, 0:D], q2[:, :, 1:DP])
                    if r < ROWS:
                        nc.sync.dma_start(out=yb[:, r], in_=sB)
                sCe = pool.tile([128, W, D], F16, tag="sCe", bufs=3,
                                name=f"sCe_{r}")
                nc.vector.tensor_add(sCe, q2[:, 0:W, 0:D], q2[:, 1:WP, 0:D])
                if r < ROWS:
                    nc.sync.dma_start(out=yc[:, r], in_=sCe)
                cd = pool.tile([128, W, D], F16, tag="cd", bufs=2,
                               name=f"cd_{r}")
                nc.vector.tensor_add(cd, sB[:, 0:W, :], sB[:, 1:WP, :])
                if r < ROWS:
                    nc.sync.dma_start(out=yd[:, r], in_=cd)
                if prev is not None:
                    # four independent odd-class tiles + stores: one slow
                    # consumer can't gate the other three buffers (the
                    # quartet-tile version let a slow store back-pressure
                    # the whole DVE stream on bandwidth-starved cores).
                    # ooo rides the idle Pool engine; it feeds only its own
                    # store, so Pool's ~5x-slower add stays off every
                    # critical path.
                    srcs = [(prev["q2"][:, 0:W, 0:D], q2[:, 0:W, 0:D]),
                            (prev["sB"][:, 0:W, :], sB[:, 0:W, :]),
                            (prev["sCe"], sCe),
                            (prev["cd"], cd)]
                    for ci, (a, b) in enumerate(srcs):
                        od = pool.tile([128, W, D], F16, tag=f"od{ci}",
                                       bufs=2, name=f"od{ci}_{r}")
                        eng = nc.gpsimd if ci == 3 else nc.vector
                        eng.tensor_add(od, a, b)
                        nc.sync.dma_start(out=yo[:, r - 1, ci], in_=od)
                prev = dict(q2=q2, sB=sB, sCe=sCe, cd=cd)

    _split_multi_waits(nc)
    return nc


def _prep_inputs(x):
    """Full [2,96,96,48,32] fp32 -> per-core in_maps [128, 7, 97, 49] fp16."""
    xt = np.ascontiguousarray(np.transpose(x, (0, 4, 1, 2, 3)))
    xh = xt.reshape(B * C, H, W, D).astype(np.float16)
    xp = np.empty((B * C, H, WP, DP), np.float16)
    xp[:, :, 0:W, 0:D] = xh
    xp[:, :, W, 0:D] = xh[:, :, W - 1, :]
    xp[:, :, :, D] = xp[:, :, :, D - 1]
    in_maps = []
    for k in range(NCORES):
        parts = []
        for half in (0, 1):
            rows = np.minimum(half * 48 + k * ROWS + np.arange(HP), H - 1)
            parts.append(xp[:, rows])  # [64, 7, 97, 49]
        xin = np.stack(parts, axis=0).reshape(128, HP, WP, DP)
        in_maps.append({"x": np.ascontiguousarray(xin)})
    return in_maps


def _assemble(results, x):
    """Per-core class planes -> full [2,192,192,96,32] f32."""
    xt = np.ascontiguousarray(
        np.transpose(np.asarray(x, np.float32), (0, 4, 1, 2, 3)))
    out = np.empty((B, TH, TW, TD, C), np.float32)
    ov = out.transpose(0, 4, 1, 2, 3)  # [2,32,192,192,96] writable view
    ov[:, :, 0::2, 0::2, 0::2] = xt    # eee class: exact copy of the input

    def put(dst, src, scale):
        if scale == 1.0:
            dst[...] = src
        else:
            np.multiply(src, np.float32(scale), out=dst, dtype=np.float32,
                        casting="unsafe")

    for k in range(NCORES):
        rk = results[k]
        ybk = np.asarray(rk["yb"]).reshape(2, B, C, ROWS, WP, D)
        yck = np.asarray(rk["yc"]).reshape(2, B, C, ROWS, W, D)
        ydk = np.asarray(rk["yd"]).reshape(2, B, C, ROWS, W, D)
        yok = np.asarray(rk["yo"]).reshape(2, B, C, ROWS, 4, W, D)
        for half in (0, 1):
            a = 2 * (48 * half + ROWS * k)
            ev = slice(a, a + 2 * ROWS, 2)
            od = slice(a + 1, a + 2 * ROWS, 2)
            put(ov[:, :, ev, 0::2, 1::2], ybk[half][:, :, :, 0:W, :], 2.0)
            put(ov[:, :, ev, 1::2, 0::2], yck[half], 2.0)
            put(ov[:, :, ev, 1::2, 1::2], ydk[half], 1.0)
            put(ov[:, :, od, 0::2, 0::2], yok[half][:, :, :, 0], 2.0)
            put(ov[:, :, od, 0::2, 1::2], yok[half][:, :, :, 1], 1.0)
            put(ov[:, :, od, 1::2, 0::2], yok[half][:, :, :, 2], 1.0)
            put(ov[:, :, od, 1::2, 1::2], yok[half][:, :, :, 3], 0.5)
    return out


def kernel(x, _trace=False):
    x = np.ascontiguousarray(np.asarray(x), dtype=np.float32)
    assert x.shape == (B, H, W, D, C), x.shape
    in_maps = _prep_inputs(x)
    nc = build_program()
    kw = {}
    if _trace:
        kw = dict(trace=True)
    res = bass_utils.run_bass_kernel_spmd(
        nc, in_maps, core_ids=list(range(NCORES)), **kw)
    out = _assemble(res.results, x)
    if _trace:
        return out, res
    return out


if __name__ == "__main__":
    rng = np.random.default_rng(0)
    x = rng.standard_normal((B, H, W, D, C), dtype=np.float32)
    y = kernel(x)
    print("out shape:", y.shape, y.dtype)
